# revision 70
# baseline (speedup 1.0000x reference)
"""Trainium2 Bass kernel for causal multi-head attention (dense transformer block).

Problem: x[2,2048,1024] -> qkv proj -> 16-head causal attention (scale 1/sqrt(1024))
         -> out proj.  8 NeuronCores.

Sharding: core c handles batch b=c//4 and head-group r=c%4 (heads 4r..4r+3).
  - qkv weights column-sharded by head group (q/k/v slices of 256 cols each)
  - attention computed fully on-core in a transposed layout:
      S^T[k,q] = K^T-chunk (stationary) x Q^T (moving) on the PE
      P = exp(S/32) with causal masking; denominator obtained by appending a
      ones-column to V so that O^T = [V|1]^T P gives sums in the last row.
  - AllGather (bf16, groups of 4 cores sharing a batch) assembles all heads'
    outputs feature-major; out-proj is column-sharded with an all-gathered
    feature dim.

Schedule (single fused stream, 4 passes of 512 query tokens each):
  - x/w load as SWDGE cast-DMAs (fp32->bf16 in the DMA, gpsimd queue);
    x is transposed on the PE (8x 53ns per 128-token tile via identity
    transpose through a PSUM bank) -- no DMA transposes at all, so the
    DMA stream is a pure load pipeline and the front is short.
  - block-0 qkv runs chunk-outer, paced to the split weight-chunk arrivals;
    later blocks' transposes + Q/K projections and the previous pass's
    out-projection are spliced as PE units into the attention job stream;
    V projections for block b splice into pass b itself (they are only
    needed at the diagonal), feeding Act-bound stretches with PE-only work.
  - attention jobs run i-major (key-tile outer, head inner) with the score
    matmul emitted two jobs ahead, so each pass's AllGather fires as soon
    as its last key tile lands; normalization is split into a per-job DVE
    part and a deferred pass-end broadcast (gpsimd for early passes, PE
    rank-1 fp32r matmul where Pool must stay clear) so the in-order PE
    queue never parks on the reciprocal chain.
  - the final pass runs heads 0,1 through their whole key loop first, so
    their half of the (per-feature-tile split) AllGather fires mid-pass;
    the tail out-projection then starts on those gathered chunks with no
    wait, overlapping the second half-gather (chunk-outer accumulation,
    evens first).
"""

import sys

sys.path.insert(0, "/opt/trn_rl_repo")

import numpy as np

import concourse.bass as bass
import concourse.bacc as bacc
import concourse.mybir as mybir
import concourse.tile as tile
from concourse.bass import ds, ts
from concourse.bass_utils import run_bass_kernel_spmd
from concourse.masks import make_upper_triangular, make_identity

F32 = mybir.dt.float32
F32R = mybir.dt.float32r
BF16 = mybir.dt.bfloat16

# ---------------------------------------------------------------- dims
BS, L, DM, H = 2, 2048, 1024, 16
HD = 64                      # head dim
NCORES = 8
GRP = 4                      # cores per batch group (head-parallel)
HLOC = H // GRP              # heads per core = 4
FLOC = HLOC * HD             # local features = 256
SCALE = 1.0 / float(np.sqrt(DM))
REPLICA_GROUPS = [[0, 1, 2, 3], [4, 5, 6, 7]]


class Cfg:
    def __init__(self, L=L, DM=DM, hloc=HLOC, hd=HD, npass=4, zero_bias=True):
        self.L, self.DM, self.HLOC, self.HD, self.NPASS = L, DM, hloc, hd, npass
        self.FLOC = hloc * hd
        self.NT = L // 128           # 128-token tiles (16)
        self.NB = L // 512           # 512-token blocks (4)
        self.NDM = DM // 128         # dmodel chunks (8)
        self.PW = L // npass         # pass width (512)
        self.NFT = self.FLOC // 128  # feature tiles for Q^T/K^T (2)
        self.scale = 1.0 / float(np.sqrt(DM))
        self.zero_bias = zero_bias
        assert self.PW == 512 and self.FLOC % 128 == 0


def build_body(nc, cfg, x, wqkv, bq, bk, bv, wo, bo, out, groups):
    """Emit the per-core program (Tile framework)."""
    NT, NB, NDM, PW, NFT = cfg.NT, cfg.NB, cfg.NDM, cfg.PW, cfg.NFT
    HLOCc, HDc, FLOCc = cfg.HLOC, cfg.HD, cfg.FLOC
    Lc, DMc = cfg.L, cfg.DM
    NPASS = cfg.NPASS
    zb = cfg.zero_bias
    tc = nc.tc

    with tc.tile_pool(name="const", bufs=1) as constp, \
         tc.tile_pool(name="persist", bufs=1) as pp:
        # ---------------- persistent SBUF tensors
        # x^T stored [p, token-tile, dm-chunk, token-in-tile]: a PE transpose
        # of one 128-token tile fills xT[:, t] with one PSUM-bank round trip
        xT = pp.tile([128, NT, NDM, 128], BF16)
        wqkvb = pp.tile([128, NDM, 3 * FLOCc], BF16)       # [wq|wk|wv] packed
        wqb = wqkvb[:, :, 0:FLOCc]
        wkb = wqkvb[:, :, FLOCc : 2 * FLOCc]
        wvb = wqkvb[:, :, 2 * FLOCc : 3 * FLOCc]
        wob = pp.tile([128, NDM, FLOCc], BF16)
        QT = pp.tile([128, NFT, Lc], BF16)                 # Q^T feature-major
        KT = pp.tile([128, NFT, Lc], BF16)
        Vb = pp.tile([128, NT, HLOCc * (HDc + 1)], BF16)   # [V | ones] per token tile
        OTs = pp.tile([128, NFT, Lc], BF16)                # attention out^T (feature-major)

        # ---------------- single PSUM pool for the whole kernel (8 banks)
        #   tag "sq" [128,512] f32 x4 bufs = 4 banks
        #     (warmup, block-0 Q/K accumulators, V/out-proj work, S tiles)
        #   tag "po" [65,512]  f32 x4 bufs = 4 banks (per-head attnV accum)
        psum_cm = tc.tile_pool(name="psum", bufs=4, space="PSUM")
        psum = psum_cm.__enter__()

        # PE warmup: junk matmuls so the p-state ramp happens on the DMA-bound
        # front, sized to end roughly when the first weight chunk lands.
        NWARM = 6
        wsrc_t = pp.tile([128, 512], BF16, name="wsrc_t")
        nc.vector.memset(wsrc_t, 0.25)
        wps = psum.tile([128, 512], F32, tag="sq", name="wps")
        for r in range(NWARM):
            nc.tensor.matmul(wps, wsrc_t[:, 0:128], wsrc_t,
                             start=(r == 0), stop=(r == NWARM - 1))
        wout_t = pp.tile([128, 512], F32, name="wout_t")
        nc.vector.tensor_copy(wout_t, wps)

        # ---------------- constants
        trimask = constp.tile([128, 128], BF16)
        make_upper_triangular(nc, trimask, val=1.0, diag=True)
        nc.vector.memset(
            Vb.rearrange("p t (h u) -> p t h u", u=HDc + 1)[:, :, :, HDc : HDc + 1], 1.0
        )
        if not zb:
            ones_r = constp.tile([1, 128], BF16)
            nc.vector.memset(ones_r, 1.0)
            bq_f = constp.tile([128, NFT], F32)
            bk_f = constp.tile([128, NFT], F32)
            bvb = constp.tile([1, FLOCc], BF16)
            bob = constp.tile([1, FLOCc], BF16)
            nc.sync.dma_start(bq_f, bq.rearrange("(f p) -> p f", p=128))
            nc.sync.dma_start(bk_f, bk.rearrange("(f p) -> p f", p=128))
            bv_st = constp.tile([1, 2 * FLOCc], F32, name="bv_st")
            nc.sync.dma_start(bv_st[:, 0:FLOCc], bv.rearrange("(a b) -> a b", a=1))
            nc.sync.dma_start(bv_st[:, FLOCc : 2 * FLOCc], bo.rearrange("(a b) -> a b", a=1))
            nc.vector.tensor_copy(bvb, bv_st[:, 0:FLOCc])
            nc.vector.tensor_copy(bob, bv_st[:, FLOCc : 2 * FLOCc])

        # ---------------- staging
        # Loads are SWDGE cast-DMAs (fp32->bf16 inside the DMA) on the gpsimd
        # (Pool) queue -- half the DMA-engine time of an fp32 load.  There are
        # NO DMA transposes: x is transposed on the PE (8x 53ns per 128-token
        # tile through a PSUM bank) during otherwise-dead front time, so the
        # DMA stream is a pure load pipeline.  w is split into the Q|K columns
        # (needed first) and the V columns.
        ident = constp.tile([128, 128], BF16, name="ident")
        make_identity(nc, ident)
        ones_c = constp.tile([1, 64], F32, name="ones_c")
        nc.vector.memset(ones_c, 1.0)
        xvb = x.rearrange("(b t p) dm -> b p t dm", p=128, t=4)  # 512-tok blocks
        wg = wqkv.rearrange("(g c p) f -> g p c f", c=4, p=128)  # 4-chunk groups
        wvv = wqkv.rearrange("(c p) f -> p c f", p=128)
        xbt = pp.tile([128, NT, DMc], BF16, name="xbt")   # token-major x (bf16)

        for t in range(4):
            nc.gpsimd.dma_start(xbt[:, t, :], xvb[0][:, t, :])
        wg2 = wqkv.rearrange("(g c p) f -> g p c f", c=2, p=128)
        for g in range(4):
            nc.gpsimd.dma_start(wqkvb[:, 2 * g : 2 * g + 2, 0:512],
                                wg2[g][:, :, 0:512])
        nc.gpsimd.dma_start(wqkvb[:, :, 2 * FLOCc : 3 * FLOCc],
                            wvv[:, :, 2 * FLOCc : 3 * FLOCc])
        for b in range(1, NB):
            nc.gpsimd.dma_start(xbt[:, 4 * b : 4 * b + 4, :], xvb[b])
        nc.gpsimd.dma_start(wob, wo.rearrange("(c p) f -> p c f", p=128))

        def emit_tr_unit(t):
            """PE-transpose one 128-token tile of x into xT."""
            tp = psum.tile([128, 512], F32, tag="sq", name="tp").bitcast(BF16)
            for c in range(NDM):
                nc.tensor.transpose(tp[:, ts(c, 128)], xbt[:, t, ts(c, 128)], ident)
            nc.vector.tensor_copy(xT[:, t, :, :], tp)

        for t in range(4):
            emit_tr_unit(t)

        # ---------------- qkv helpers
        def qk_copy(dst, src, bias_col):
            if zb:
                nc.vector.tensor_copy(dst, src)
            else:
                nc.scalar.activation(dst, src,
                                     mybir.ActivationFunctionType.Identity,
                                     bias=bias_col)

        def emit_v_unit(tt):
            """Project V for one 128-token tile (tokens on partitions)."""
            psv_full = psum.tile([128, 512], F32, tag="sq", name="psv")
            psv = psv_full[:, 0:FLOCc]
            for c in range(NDM):
                nc.tensor.matmul(psv, xT[:, tt, c, :], wvb[:, c, :],
                                 start=(c == 0), stop=(zb and c == NDM - 1))
            if not zb:
                nc.tensor.matmul(psv, ones_r, bvb, start=False, stop=True)
            nc.vector.tensor_copy(
                Vb[:, tt, :].rearrange("p (h u) -> p h u", u=HDc + 1)[:, :, 0:HDc],
                psv.rearrange("p (h d) -> p h d", d=HDc),
            )

        def emit_qk_unit(tb, ft, which):
            """Q or K projection for one 512-token block, one feature tile."""
            wsel = wqb if which == "q" else wkb
            dstT = QT if which == "q" else KT
            qk = psum.tile([128, 512], F32, tag="sq", name="qk")
            for c in range(NDM):
                nc.tensor.matmul(qk, wsel[:, c, ts(ft, 128)],
                                 xT[:, 4 * tb : 4 * tb + 4, c, :],
                                 start=(c == 0), stop=(c == NDM - 1))
            bias_col = None if zb else (bq_f if which == "q" else bk_f)[:, ft : ft + 1]
            qk_copy(dstT[:, ft, ts(tb, 512)], qk, bias_col)

        # ---------------- block-0 qkv: chunk-outer, paced to w-chunk arrival
        qk0 = {}
        for which in ("q", "k"):
            for ft in range(NFT):
                qk0[(which, ft)] = psum.tile([128, 512], F32, tag="sq",
                                             name=f"qk0_{which}{ft}")
        for c in range(NDM):
            for which in ("q", "k"):
                wsel = wqb if which == "q" else wkb
                for ft in range(NFT):
                    nc.tensor.matmul(qk0[(which, ft)],
                                     wsel[:, c, ts(ft, 128)], xT[:, 0:4, c, :],
                                     start=(c == 0), stop=(c == NDM - 1))
        for which in ("q", "k"):
            for ft in range(NFT):
                bias_col = None if zb else (bq_f if which == "q" else bk_f)[:, ft : ft + 1]
                qk_copy((QT if which == "q" else KT)[:, ft, 0:512], qk0[(which, ft)],
                        bias_col)
        for tt in range(4):
            emit_v_unit(tt)

        # ---------------- attention + allgather + out projection
        with tc.tile_pool(name="pbuf", bufs=6) as pbp, \
             tc.tile_pool(name="nrm", bufs=3) as nrm, \
             tc.tile_pool(name="of", bufs=2) as ofp, \
             tc.tile_pool(name="osb", bufs=2) as osbp, \
             tc.tile_pool(name="dram", bufs=2, space="DRAM") as dramp:

            all_jobs = [(p, i, h)
                        for p in range(NPASS - 1)
                        for i in range(4 * p + 4)
                        for h in range(HLOCc)]
            # final pass: heads 0,1 run their whole key loop first so their
            # half of the AllGather fires mid-pass
            pL = NPASS - 1
            all_jobs += [(pL, i, h)
                         for pair in ((0, 1), (2, 3))
                         for i in range(4 * pL + 4)
                         for h in pair]

            def emit_scores(p, i, h):
                hf, hp = h // 2, h % 2
                al = max(0, 128 * i - PW * p)
                S = psum.tile([128, 512], F32, tag="sq", name="S")
                nc.tensor.matmul(
                    S[:, ds(al, 512 - al)],
                    KT[64 * hp : 64 * hp + 64, hf, ts(i, 128)],
                    QT[64 * hp : 64 * hp + 64, hf, ds(PW * p + al, 512 - al)],
                    start=True, stop=True,
                )
                return S, al

            def emit_agather(p, store=True):
                """Store OTs super p and kick the AllGather; load OF chunks."""
                q0 = PW * p
                if store:
                    ag_in = dramp.tile([NFT * 128, 512], BF16, tag="agin",
                                       name="ag_in")
                    nc.sync.dma_start(ag_in.rearrange("(t p) q -> p t q", p=128),
                                      OTs[:, :, ds(q0, 512)])
                else:
                    ag_in = ag_in_last
                ag_out = dramp.tile([GRP * NFT * 128, 512], BF16, tag="agout",
                                    name="ag_out")
                nc.gpsimd.collective_compute(
                    "AllGather",
                    mybir.AluOpType.bypass,
                    ins=[ag_in.opt()],
                    outs=[ag_out.opt()],
                    replica_groups=groups,
                )
                OF = ofp.tile([128, NDM, 512], BF16, tag="of", name="OF")
                agv = ag_out.rearrange("(c p) q -> c p q", p=128)
                for c in range(NDM):
                    nc.sync.dma_start(OF[:, c, :], agv[c])
                return OF

            OF_pending = {}   # p -> OF tile
            osb_pending = {}  # p -> osb tile

            def emit_oproj_unit(p, ttl):
                """Out-projection for 128 tokens of super p (spliced PE unit)."""
                OF = OF_pending[p]
                q0 = PW * p
                if ttl == 0:
                    osb_pending[p] = osbp.tile([128, 4, FLOCc], F32, tag="osb",
                                               name="osb")
                osb = osb_pending[p]
                pout_full = psum.tile([128, 512], F32, tag="sq", name="pout")
                pout = pout_full[:, 0:FLOCc]
                for c in range(NDM):
                    nc.tensor.matmul(pout, OF[:, c, ts(ttl, 128)], wob[:, c, :],
                                     start=(c == 0), stop=(zb and c == NDM - 1))
                if not zb:
                    nc.tensor.matmul(pout, ones_r, bob, start=False, stop=True)
                nc.vector.tensor_copy(osb[:, ttl, :], pout)
                outv = out[ds(q0, 512), :].rearrange("(t p) f -> p t f", p=128)
                if ttl == 1:
                    nc.sync.dma_start(outv[:, 0:2, :], osb[:, 0:2, :])
                elif ttl == 3:
                    nc.sync.dma_start(outv[:, 2:4, :], osb[:, 2:4, :])

            # ---- build the spliced unit schedule per pass.
            # pass p carries: qkv units for block p+1, out-proj units for p-1.
            pass_units = {p: [] for p in range(NPASS)}
            for p in range(NPASS - 1):
                tb = p + 1
                u = []
                for tt in range(4 * tb, 4 * tb + 4):
                    u.append(lambda tt=tt: emit_tr_unit(tt))
                for ft in range(NFT):
                    u.append(lambda tb=tb, ft=ft: emit_qk_unit(tb, ft, "q"))
                if tb == 1:
                    for ft in range(NFT):
                        u.append(lambda ft=ft: emit_qk_unit(1, ft, "k"))
                pass_units[p].extend(u)
            # K for the final block is first needed at key tile 12 (pair-order
            # job index 24): splice it into the Act-bound final pass instead
            k3_sched = {14: [lambda: emit_qk_unit(NPASS - 1, 0, "k")],
                        18: [lambda: emit_qk_unit(NPASS - 1, 1, "k")]}
            # likewise K for block 2 is first needed at pass-2 key tile 8
            # (job index 32): splice into pass 2 itself
            k2_sched = {20: [lambda: emit_qk_unit(NPASS - 2, 0, "k")],
                        24: [lambda: emit_qk_unit(NPASS - 2, 1, "k")]}

            # V units for block b are first used at pass b's own diagonal
            # tiles (i = 4b+k), so splice them into pass b shortly before use:
            # this feeds Act-bound attention stretches with PE-only work.
            v_sched = {p: {} for p in range(NPASS)}
            for p in range(1, NPASS):
                for k in range(4):
                    vidx = 2 + 5 * k
                    v_sched[p].setdefault(vidx, []).append(
                        lambda tt=4 * p + k: emit_v_unit(tt))
            # out-proj splice indices are appended at runtime (see loop below).

            po_cur = {}       # h -> psum tile for current pass
            osnaps = {}       # h -> osnap tile (pass-end normalization)
            recs = {}         # h -> [1,512] reciprocal tile

            rbs_pend = {}     # h -> broadcast reciprocal (emitted per job)

            def flush_norm(p, hs):
                """Deferred normalization part 2 for the given heads."""
                for h2 in hs:
                    nc.vector.tensor_mul(
                        OTs[64 * (h2 % 2) : 64 * (h2 % 2) + 64, h2 // 2,
                            ds(PW * p, 512)],
                        osnaps[h2][0:HDc, :],
                        rbs_pend[h2],
                    )
            ag_in_last = dramp.tile([NFT * 128, 512], BF16, tag="aginL",
                                    name="ag_in_last", bufs=1)
            ag_outA = dramp.tile([GRP * 128, 512], BF16, tag="agoutA",
                                 name="ag_outA", bufs=1)
            ag_outB = dramp.tile([GRP * 128, 512], BF16, tag="agoutB",
                                 name="ag_outB", bufs=1)
            OFL3 = ofp.tile([128, NDM, 512], BF16, tag="of", name="OFL3")

            for p in range(NPASS):
                npjobs = 4 * (4 * p + 4)
                base = sum(4 * (4 * q + 4) for q in range(p))
                # splice schedule: unit k fires before job index sched[k]
                units = list(pass_units[p])
                if p >= 1:
                    # out-proj of pass p-1: not before ~1/3 through the stream
                    # (AllGather must have completed)
                    op_units = [lambda pp_=p - 1, ttl=ttl: emit_oproj_unit(pp_, ttl)
                                for ttl in range(4)]
                else:
                    op_units = []
                qkv_sched = [max(1, (k + 1) * npjobs // (len(units) + 1))
                             for k in range(len(units))]
                op_start = max(1, 3 * npjobs // 4)
                op_sched = [min(npjobs - 1, op_start + k * max(1, npjobs // 8))
                            for k in range(len(op_units))]
                sched = {}
                for k, u in enumerate(units):
                    sched.setdefault(qkv_sched[k], []).append(u)
                for k, u in enumerate(op_units):
                    sched.setdefault(op_sched[k], []).append(u)
                for idx2, us in v_sched[p].items():
                    sched.setdefault(idx2, []).extend(us)
                if p == NPASS - 1:
                    for idx2, us in k3_sched.items():
                        sched.setdefault(idx2, []).extend(us)
                if p == NPASS - 2:
                    for idx2, us in k2_sched.items():
                        sched.setdefault(idx2, []).extend(us)


                for idx in range(npjobs):
                    gidx = base + idx
                    _, i, h = all_jobs[gidx]
                    hf, hp = h // 2, h % 2
                    ilast = 4 * p + 3
                    for u in sched.get(idx, []):
                        u()
                    if gidx == 0:
                        S_cur, al_cur = emit_scores(*all_jobs[0])
                        S_n1, al_n1 = emit_scores(*all_jobs[1])
                    S, al = S_cur, al_cur
                    S_cur, al_cur = S_n1, al_n1
                    if gidx + 2 < len(all_jobs):
                        S_n1, al_n1 = emit_scores(*all_jobs[gidx + 2])
                    # P = exp(S/32), causal-masked on the diagonal block
                    P = pbp.tile([128, 512], BF16, tag="ptile", name="P")
                    nc.scalar.activation(
                        P[:, ds(al, 512 - al)],
                        S[:, ds(al, 512 - al)],
                        mybir.ActivationFunctionType.Exp,
                        scale=float(cfg.scale),
                    )
                    if i >= 4 * p:  # diagonal block
                        nc.vector.tensor_mul(P[:, ds(al, 128)], P[:, ds(al, 128)],
                                             trimask)
                    if i == 0:
                        po_cur[h] = psum.tile([HDc + 1, 512], F32, tag="po",
                                              name="po")
                    po = po_cur[h]
                    # attnV accumulation, causally trimmed (partial-stop on the
                    # last key tile: the closing matmul covers only [al, 512))
                    nc.tensor.matmul(
                        po[:, ds(al, 512 - al)],
                        Vb[:, i, ds((HDc + 1) * h, HDc + 1)],
                        P[:, ds(al, 512 - al)],
                        start=(i == 0), stop=(i == ilast),
                    )
                    if i == ilast:
                        # normalization, part 1 (DVE only -- the PE part is
                        # deferred to the pass end so the in-order PE queue
                        # never parks on the reciprocal chain).  The mul
                        # reads po (PSUM) directly: rb is SBUF, so only one
                        # PSUM input -- no snapshot copy needed.
                        osrc = po
                        rec = nrm.tile([1, 512], F32R, tag="rec",
                                       name="rec", bufs=6)
                        with nc.allow_low_precision(reason="f32r is 32-bit"):
                            nc.vector.reciprocal(rec, osrc[HDc : HDc + 1, :])
                        recs[h] = rec
                        osnaps[h] = osrc
                        rbj = nrm.tile([64, 512], F32, tag="rb", name="rb",
                                       bufs=6)
                        nc.gpsimd.partition_broadcast(rbj, rec.bitcast(F32))
                        rbs_pend[h] = rbj
                    if p == NPASS - 1 and idx == npjobs // 2 - 1:
                        # heads 0,1 done: normalize them, store + gather
                        # feature tile 0 while heads 2,3 still attend
                        flush_norm(p, (0, 1))
                        nc.sync.dma_start(
                            ag_in_last[0:128, :], OTs[:, 0, ds(PW * p, 512)])
                        nc.gpsimd.collective_compute(
                            "AllGather", mybir.AluOpType.bypass,
                            ins=[ag_in_last[0:128, :].opt()],
                            outs=[ag_outA.opt()], replica_groups=groups)
                        agvA = ag_outA.rearrange("(r p) q -> r p q", p=128)
                        for c in (0, 2, 4, 6):
                            nc.sync.dma_start(OFL3[:, c, :], agvA[c // 2])
                if p == NPASS - 1:
                    flush_norm(p, (2, 3))
                    nc.sync.dma_start(
                        ag_in_last[128:256, :], OTs[:, 1, ds(PW * p, 512)])
                    nc.gpsimd.collective_compute(
                        "AllGather", mybir.AluOpType.bypass,
                        ins=[ag_in_last[128:256, :].opt()],
                        outs=[ag_outB.opt()], replica_groups=groups)
                    agvB = ag_outB.rearrange("(r p) q -> r p q", p=128)
                    for c in (1, 3, 5, 7):
                        nc.sync.dma_start(OFL3[:, c, :], agvB[c // 2])
                    OF_pending[p] = OFL3
                else:
                    flush_norm(p, tuple(range(HLOCc)))
                    # pass p complete: fire its AllGather (non-PE queues)
                    OF_pending[p] = emit_agather(p)

            # final pass's out-projection (unavoidable tail): chunk-outer so
            # each matmul only waits for its own gathered-feature chunk
            OFt = OF_pending[NPASS - 1]
            q0t = PW * (NPASS - 1)
            osbt = osbp.tile([128, 4, FLOCc], F32, tag="osb", name="osbt")
            poutt = [psum.tile([128, 512], F32, tag="sq", name="poutt")
                     for _ in range(4)]
            outvt = out[ds(q0t, 512), :].rearrange("(t p) f -> p t f", p=128)
            tail_cs = [0, 2, 4, 6, 1, 3, 5, 7]
            for ci, c in enumerate(tail_cs):
                for ttl in range(4):
                    nc.tensor.matmul(poutt[ttl][:, 0:FLOCc],
                                     OFt[:, c, ts(ttl, 128)], wob[:, c, :],
                                     start=(ci == 0),
                                     stop=(zb and ci == NDM - 1))
            for ttl in range(4):
                if not zb:
                    nc.tensor.matmul(poutt[ttl][:, 0:FLOCc], ones_r, bob,
                                     start=False, stop=True)
                nc.vector.tensor_copy(osbt[:, ttl, :], poutt[ttl][:, 0:FLOCc])
                if ttl == 1:
                    nc.sync.dma_start(outvt[:, 0:2, :], osbt[:, 0:2, :])
                elif ttl == 3:
                    nc.sync.dma_start(outvt[:, 2:4, :], osbt[:, 2:4, :])

        psum_cm.__exit__(None, None, None)


def make_program(cfg=None, groups=None, unroll=1):
    cfg = cfg or Cfg()
    groups = groups or REPLICA_GROUPS
    nc = bacc.Bacc("TRN2", target_bir_lowering=False, debug=False, num_devices=NCORES)
    x = nc.dram_tensor("x", [cfg.L, cfg.DM], F32, kind="ExternalInput").ap()
    wqkv = nc.dram_tensor("wqkv", [cfg.DM, 3 * cfg.FLOC], F32, kind="ExternalInput").ap()
    bq = nc.dram_tensor("bq", [cfg.FLOC], F32, kind="ExternalInput").ap()
    bk = nc.dram_tensor("bk", [cfg.FLOC], F32, kind="ExternalInput").ap()
    bv = nc.dram_tensor("bv", [cfg.FLOC], F32, kind="ExternalInput").ap()
    wo = nc.dram_tensor("wo", [cfg.DM, cfg.FLOC], F32, kind="ExternalInput").ap()
    bo = nc.dram_tensor("bo", [cfg.FLOC], F32, kind="ExternalInput").ap()
    out = nc.dram_tensor("out", [cfg.L, cfg.FLOC], F32, kind="ExternalOutput").ap()
    with tile.TileContext(nc) as tc:
        nc.tc = tc
        for _ in range(unroll):
            build_body(nc, cfg, x, wqkv, bq, bk, bv, wo, bo, out, groups)
    nc.compile()
    return nc


def shard_inputs(x, w_qkv, b_qkv, w_out, b_out, cfg=None):
    """Full inputs -> list of 8 per-core input dicts."""
    cfg = cfg or Cfg()
    FL = cfg.FLOC
    DMF = cfg.DM
    in_maps = []
    for c in range(NCORES):
        b, r = divmod(c, GRP)
        q0 = r * FL
        in_maps.append({
            "x": np.ascontiguousarray(x[b]),
            "wqkv": np.ascontiguousarray(np.concatenate([
                w_qkv[:, q0 : q0 + FL],
                w_qkv[:, DMF + q0 : DMF + q0 + FL],
                w_qkv[:, 2 * DMF + q0 : 2 * DMF + q0 + FL],
            ], axis=1)),
            "bq": np.ascontiguousarray(b_qkv[q0 : q0 + FL]),
            "bk": np.ascontiguousarray(b_qkv[DMF + q0 : DMF + q0 + FL]),
            "bv": np.ascontiguousarray(b_qkv[2 * DMF + q0 : 2 * DMF + q0 + FL]),
            "wo": np.ascontiguousarray(w_out[:, q0 : q0 + FL]),
            "bo": np.ascontiguousarray(b_out[q0 : q0 + FL]),
        })
    return in_maps


def gather_output(results, cfg=None):
    cfg = cfg or Cfg()
    FL = cfg.FLOC
    out = np.empty((BS, cfg.L, cfg.DM), np.float32)
    for c in range(NCORES):
        b, r = divmod(c, GRP)
        out[b, :, r * FL : (r + 1) * FL] = results[c]["out"]
    return out


_PROGRAMS = {}


def _get_program(zero_bias):
    if zero_bias not in _PROGRAMS:
        _PROGRAMS[zero_bias] = make_program(Cfg(zero_bias=zero_bias))
    return _PROGRAMS[zero_bias]


def kernel(x, w_qkv, b_qkv, w_out, b_out):
    x = np.asarray(x, np.float32)
    w_qkv = np.asarray(w_qkv, np.float32)
    b_qkv = np.asarray(b_qkv, np.float32)
    w_out = np.asarray(w_out, np.float32)
    b_out = np.asarray(b_out, np.float32)
    zero_bias = bool(not b_qkv.any() and not b_out.any())
    nc = _get_program(zero_bias)
    in_maps = shard_inputs(x, w_qkv, b_qkv, w_out, b_out)
    res = run_bass_kernel_spmd(nc, in_maps, list(range(NCORES)))
    return gather_output(res.results)


# revision 71
# speedup vs baseline: 1.0002x; 1.0002x over previous
"""Trainium2 Bass kernel for causal multi-head attention (dense transformer block).

Problem: x[2,2048,1024] -> qkv proj -> 16-head causal attention (scale 1/sqrt(1024))
         -> out proj.  8 NeuronCores.

Sharding: core c handles batch b=c//4 and head-group r=c%4 (heads 4r..4r+3).
  - qkv weights column-sharded by head group (q/k/v slices of 256 cols each)
  - attention computed fully on-core in a transposed layout:
      S^T[k,q] = K^T-chunk (stationary) x Q^T (moving) on the PE
      P = exp(S/32) with causal masking; denominator obtained by appending a
      ones-column to V so that O^T = [V|1]^T P gives sums in the last row.
  - AllGather (bf16, groups of 4 cores sharing a batch) assembles all heads'
    outputs feature-major; out-proj is column-sharded with an all-gathered
    feature dim.

Schedule (single fused stream, 4 passes of 512 query tokens each):
  - x/w load as SWDGE cast-DMAs (fp32->bf16 in the DMA, gpsimd queue);
    x is transposed on the PE (8x 53ns per 128-token tile via identity
    transpose through a PSUM bank) -- no DMA transposes at all, so the
    DMA stream is a pure load pipeline and the front is short.
  - block-0 qkv runs chunk-outer, paced to the split weight-chunk arrivals;
    later blocks' transposes + Q/K projections and the previous pass's
    out-projection are spliced as PE units into the attention job stream;
    V projections for block b splice into pass b itself (they are only
    needed at the diagonal), feeding Act-bound stretches with PE-only work.
  - attention jobs run i-major (key-tile outer, head inner) with the score
    matmul emitted two jobs ahead, so each pass's AllGather fires as soon
    as its last key tile lands; normalization is split into a per-job DVE
    part and a deferred pass-end broadcast (gpsimd for early passes, PE
    rank-1 fp32r matmul where Pool must stay clear) so the in-order PE
    queue never parks on the reciprocal chain.
  - the final pass runs heads 0,1 through their whole key loop first, so
    their half of the (per-feature-tile split) AllGather fires mid-pass;
    the tail out-projection then starts on those gathered chunks with no
    wait, overlapping the second half-gather (chunk-outer accumulation,
    evens first).
"""

import sys

sys.path.insert(0, "/opt/trn_rl_repo")

import numpy as np

import concourse.bass as bass
import concourse.bacc as bacc
import concourse.mybir as mybir
import concourse.tile as tile
from concourse.bass import ds, ts
from concourse.bass_utils import run_bass_kernel_spmd
from concourse.masks import make_upper_triangular, make_identity

F32 = mybir.dt.float32
F32R = mybir.dt.float32r
BF16 = mybir.dt.bfloat16

# ---------------------------------------------------------------- dims
BS, L, DM, H = 2, 2048, 1024, 16
HD = 64                      # head dim
NCORES = 8
GRP = 4                      # cores per batch group (head-parallel)
HLOC = H // GRP              # heads per core = 4
FLOC = HLOC * HD             # local features = 256
SCALE = 1.0 / float(np.sqrt(DM))
REPLICA_GROUPS = [[0, 1, 2, 3], [4, 5, 6, 7]]


class Cfg:
    def __init__(self, L=L, DM=DM, hloc=HLOC, hd=HD, npass=4, zero_bias=True):
        self.L, self.DM, self.HLOC, self.HD, self.NPASS = L, DM, hloc, hd, npass
        self.FLOC = hloc * hd
        self.NT = L // 128           # 128-token tiles (16)
        self.NB = L // 512           # 512-token blocks (4)
        self.NDM = DM // 128         # dmodel chunks (8)
        self.PW = L // npass         # pass width (512)
        self.NFT = self.FLOC // 128  # feature tiles for Q^T/K^T (2)
        self.scale = 1.0 / float(np.sqrt(DM))
        self.zero_bias = zero_bias
        assert self.PW == 512 and self.FLOC % 128 == 0


def build_body(nc, cfg, x, wqkv, bq, bk, bv, wo, bo, out, groups):
    """Emit the per-core program (Tile framework)."""
    NT, NB, NDM, PW, NFT = cfg.NT, cfg.NB, cfg.NDM, cfg.PW, cfg.NFT
    HLOCc, HDc, FLOCc = cfg.HLOC, cfg.HD, cfg.FLOC
    Lc, DMc = cfg.L, cfg.DM
    NPASS = cfg.NPASS
    zb = cfg.zero_bias
    tc = nc.tc

    with tc.tile_pool(name="const", bufs=1) as constp, \
         tc.tile_pool(name="persist", bufs=1) as pp:
        # ---------------- persistent SBUF tensors
        # x^T stored [p, token-tile, dm-chunk, token-in-tile]: a PE transpose
        # of one 128-token tile fills xT[:, t] with one PSUM-bank round trip
        xT = pp.tile([128, NT, NDM, 128], BF16)
        wqkvb = pp.tile([128, NDM, 3 * FLOCc], BF16)       # [wq|wk|wv] packed
        wqb = wqkvb[:, :, 0:FLOCc]
        wkb = wqkvb[:, :, FLOCc : 2 * FLOCc]
        wvb = wqkvb[:, :, 2 * FLOCc : 3 * FLOCc]
        wob = pp.tile([128, NDM, FLOCc], BF16)
        QT = pp.tile([128, NFT, Lc], BF16)                 # Q^T feature-major
        KT = pp.tile([128, NFT, Lc], BF16)
        Vb = pp.tile([128, NT, HLOCc * (HDc + 1)], BF16)   # [V | ones] per token tile
        OTs = pp.tile([128, NFT, Lc], BF16)                # attention out^T (feature-major)

        # ---------------- single PSUM pool for the whole kernel (8 banks)
        #   tag "sq" [128,512] f32 x4 bufs = 4 banks
        #     (warmup, block-0 Q/K accumulators, V/out-proj work, S tiles)
        #   tag "po" [65,512]  f32 x4 bufs = 4 banks (per-head attnV accum)
        psum_cm = tc.tile_pool(name="psum", bufs=4, space="PSUM")
        psum = psum_cm.__enter__()

        # PE warmup: junk matmuls so the p-state ramp happens on the DMA-bound
        # front, sized to end roughly when the first weight chunk lands.
        NWARM = 6
        wsrc_t = pp.tile([128, 512], BF16, name="wsrc_t")
        nc.vector.memset(wsrc_t, 0.25)
        wps = psum.tile([128, 512], F32, tag="sq", name="wps")
        for r in range(NWARM):
            nc.tensor.matmul(wps, wsrc_t[:, 0:128], wsrc_t,
                             start=(r == 0), stop=(r == NWARM - 1))
        wout_t = pp.tile([128, 512], F32, name="wout_t")
        nc.vector.tensor_copy(wout_t, wps)

        # ---------------- constants
        trimask = constp.tile([128, 128], BF16)
        make_upper_triangular(nc, trimask, val=1.0, diag=True)
        nc.vector.memset(
            Vb.rearrange("p t (h u) -> p t h u", u=HDc + 1)[:, :, :, HDc : HDc + 1], 1.0
        )
        if not zb:
            ones_r = constp.tile([1, 128], BF16)
            nc.vector.memset(ones_r, 1.0)
            bq_f = constp.tile([128, NFT], F32)
            bk_f = constp.tile([128, NFT], F32)
            bvb = constp.tile([1, FLOCc], BF16)
            bob = constp.tile([1, FLOCc], BF16)
            nc.sync.dma_start(bq_f, bq.rearrange("(f p) -> p f", p=128))
            nc.sync.dma_start(bk_f, bk.rearrange("(f p) -> p f", p=128))
            bv_st = constp.tile([1, 2 * FLOCc], F32, name="bv_st")
            nc.sync.dma_start(bv_st[:, 0:FLOCc], bv.rearrange("(a b) -> a b", a=1))
            nc.sync.dma_start(bv_st[:, FLOCc : 2 * FLOCc], bo.rearrange("(a b) -> a b", a=1))
            nc.vector.tensor_copy(bvb, bv_st[:, 0:FLOCc])
            nc.vector.tensor_copy(bob, bv_st[:, FLOCc : 2 * FLOCc])

        # ---------------- staging
        # Loads are SWDGE cast-DMAs (fp32->bf16 inside the DMA) on the gpsimd
        # (Pool) queue -- half the DMA-engine time of an fp32 load.  There are
        # NO DMA transposes: x is transposed on the PE (8x 53ns per 128-token
        # tile through a PSUM bank) during otherwise-dead front time, so the
        # DMA stream is a pure load pipeline.  w is split into the Q|K columns
        # (needed first) and the V columns.
        ident = constp.tile([128, 128], BF16, name="ident")
        make_identity(nc, ident)
        ones_c = constp.tile([1, 64], F32, name="ones_c")
        nc.vector.memset(ones_c, 1.0)
        xvb = x.rearrange("(b t p) dm -> b p t dm", p=128, t=4)  # 512-tok blocks
        wg = wqkv.rearrange("(g c p) f -> g p c f", c=4, p=128)  # 4-chunk groups
        wvv = wqkv.rearrange("(c p) f -> p c f", p=128)
        xbt = pp.tile([128, NT, DMc], BF16, name="xbt")   # token-major x (bf16)

        for t in range(4):
            nc.gpsimd.dma_start(xbt[:, t, :], xvb[0][:, t, :])
        wg2 = wqkv.rearrange("(g c p) f -> g p c f", c=2, p=128)
        for g in range(4):
            nc.gpsimd.dma_start(wqkvb[:, 2 * g : 2 * g + 2, 0:512],
                                wg2[g][:, :, 0:512])
        nc.gpsimd.dma_start(wqkvb[:, :, 2 * FLOCc : 3 * FLOCc],
                            wvv[:, :, 2 * FLOCc : 3 * FLOCc])
        for b in range(1, NB):
            nc.gpsimd.dma_start(xbt[:, 4 * b : 4 * b + 4, :], xvb[b])
        nc.gpsimd.dma_start(wob, wo.rearrange("(c p) f -> p c f", p=128))

        def emit_tr_unit(t):
            """PE-transpose one 128-token tile of x into xT."""
            tp = psum.tile([128, 512], F32, tag="sq", name="tp").bitcast(BF16)
            for c in range(NDM):
                nc.tensor.transpose(tp[:, ts(c, 128)], xbt[:, t, ts(c, 128)], ident)
            nc.vector.tensor_copy(xT[:, t, :, :], tp)

        for t in range(4):
            emit_tr_unit(t)

        # ---------------- qkv helpers
        def qk_copy(dst, src, bias_col):
            if zb:
                nc.vector.tensor_copy(dst, src)
            else:
                nc.scalar.activation(dst, src,
                                     mybir.ActivationFunctionType.Identity,
                                     bias=bias_col)

        def emit_v_unit(tt):
            """Project V for one 128-token tile (tokens on partitions)."""
            psv_full = psum.tile([128, 512], F32, tag="sq", name="psv")
            psv = psv_full[:, 0:FLOCc]
            for c in range(NDM):
                nc.tensor.matmul(psv, xT[:, tt, c, :], wvb[:, c, :],
                                 start=(c == 0), stop=(zb and c == NDM - 1))
            if not zb:
                nc.tensor.matmul(psv, ones_r, bvb, start=False, stop=True)
            nc.vector.tensor_copy(
                Vb[:, tt, :].rearrange("p (h u) -> p h u", u=HDc + 1)[:, :, 0:HDc],
                psv.rearrange("p (h d) -> p h d", d=HDc),
            )

        def emit_qk_unit(tb, ft, which):
            """Q or K projection for one 512-token block, one feature tile."""
            wsel = wqb if which == "q" else wkb
            dstT = QT if which == "q" else KT
            qk = psum.tile([128, 512], F32, tag="sq", name="qk")
            for c in range(NDM):
                nc.tensor.matmul(qk, wsel[:, c, ts(ft, 128)],
                                 xT[:, 4 * tb : 4 * tb + 4, c, :],
                                 start=(c == 0), stop=(c == NDM - 1))
            bias_col = None if zb else (bq_f if which == "q" else bk_f)[:, ft : ft + 1]
            qk_copy(dstT[:, ft, ts(tb, 512)], qk, bias_col)

        # ---------------- block-0 qkv: chunk-outer, paced to w-chunk arrival
        qk0 = {}
        for which in ("q", "k"):
            for ft in range(NFT):
                qk0[(which, ft)] = psum.tile([128, 512], F32, tag="sq",
                                             name=f"qk0_{which}{ft}")
        for c in range(NDM):
            for which in ("q", "k"):
                wsel = wqb if which == "q" else wkb
                for ft in range(NFT):
                    nc.tensor.matmul(qk0[(which, ft)],
                                     wsel[:, c, ts(ft, 128)], xT[:, 0:4, c, :],
                                     start=(c == 0), stop=(c == NDM - 1))
        for which in ("q", "k"):
            for ft in range(NFT):
                bias_col = None if zb else (bq_f if which == "q" else bk_f)[:, ft : ft + 1]
                qk_copy((QT if which == "q" else KT)[:, ft, 0:512], qk0[(which, ft)],
                        bias_col)
        for tt in range(4):
            emit_v_unit(tt)

        # ---------------- attention + allgather + out projection
        with tc.tile_pool(name="pbuf", bufs=6) as pbp, \
             tc.tile_pool(name="nrm", bufs=3) as nrm, \
             tc.tile_pool(name="of", bufs=2) as ofp, \
             tc.tile_pool(name="osb", bufs=2) as osbp, \
             tc.tile_pool(name="dram", bufs=2, space="DRAM") as dramp:

            all_jobs = [(p, i, h)
                        for p in range(NPASS - 1)
                        for i in range(4 * p + 4)
                        for h in range(HLOCc)]
            # final pass: heads 0,1 run their whole key loop first so their
            # half of the AllGather fires mid-pass
            pL = NPASS - 1
            all_jobs += [(pL, i, h)
                         for pair in ((0, 1), (2, 3))
                         for i in range(4 * pL + 4)
                         for h in pair]

            def emit_scores(p, i, h):
                hf, hp = h // 2, h % 2
                al = max(0, 128 * i - PW * p)
                S = psum.tile([128, 512], F32, tag="sq", name="S")
                nc.tensor.matmul(
                    S[:, ds(al, 512 - al)],
                    KT[64 * hp : 64 * hp + 64, hf, ts(i, 128)],
                    QT[64 * hp : 64 * hp + 64, hf, ds(PW * p + al, 512 - al)],
                    start=True, stop=True,
                )
                return S, al

            def emit_agather(p, store=True):
                """Store OTs super p and kick the AllGather; load OF chunks."""
                q0 = PW * p
                if store:
                    ag_in = dramp.tile([NFT * 128, 512], BF16, tag="agin",
                                       name="ag_in")
                    nc.sync.dma_start(ag_in.rearrange("(t p) q -> p t q", p=128),
                                      OTs[:, :, ds(q0, 512)])
                else:
                    ag_in = ag_in_last
                ag_out = dramp.tile([GRP * NFT * 128, 512], BF16, tag="agout",
                                    name="ag_out")
                nc.gpsimd.collective_compute(
                    "AllGather",
                    mybir.AluOpType.bypass,
                    ins=[ag_in.opt()],
                    outs=[ag_out.opt()],
                    replica_groups=groups,
                )
                OF = ofp.tile([128, NDM, 512], BF16, tag="of", name="OF")
                agv = ag_out.rearrange("(c p) q -> c p q", p=128)
                for c in range(NDM):
                    nc.sync.dma_start(OF[:, c, :], agv[c])
                return OF

            OF_pending = {}   # p -> OF tile
            osb_pending = {}  # p -> osb tile

            def emit_oproj_unit(p, ttl):
                """Out-projection for 128 tokens of super p (spliced PE unit)."""
                OF = OF_pending[p]
                q0 = PW * p
                if ttl == 0:
                    osb_pending[p] = osbp.tile([128, 4, FLOCc], F32, tag="osb",
                                               name="osb")
                osb = osb_pending[p]
                pout_full = psum.tile([128, 512], F32, tag="sq", name="pout")
                pout = pout_full[:, 0:FLOCc]
                for c in range(NDM):
                    nc.tensor.matmul(pout, OF[:, c, ts(ttl, 128)], wob[:, c, :],
                                     start=(c == 0), stop=(zb and c == NDM - 1))
                if not zb:
                    nc.tensor.matmul(pout, ones_r, bob, start=False, stop=True)
                nc.vector.tensor_copy(osb[:, ttl, :], pout)
                outv = out[ds(q0, 512), :].rearrange("(t p) f -> p t f", p=128)
                if ttl == 1:
                    nc.sync.dma_start(outv[:, 0:2, :], osb[:, 0:2, :])
                elif ttl == 3:
                    nc.sync.dma_start(outv[:, 2:4, :], osb[:, 2:4, :])

            # ---- build the spliced unit schedule per pass.
            # pass p carries: qkv units for block p+1, out-proj units for p-1.
            pass_units = {p: [] for p in range(NPASS)}
            for p in range(NPASS - 1):
                tb = p + 1
                u = []
                for tt in range(4 * tb, 4 * tb + 4):
                    u.append(lambda tt=tt: emit_tr_unit(tt))
                for ft in range(NFT):
                    u.append(lambda tb=tb, ft=ft: emit_qk_unit(tb, ft, "q"))
                if tb == 1:
                    for ft in range(NFT):
                        u.append(lambda ft=ft: emit_qk_unit(1, ft, "k"))
                pass_units[p].extend(u)
            # K for the final block is first needed at key tile 12 (pair-order
            # job index 24): splice it into the Act-bound final pass instead
            k3_sched = {14: [lambda: emit_qk_unit(NPASS - 1, 0, "k")],
                        18: [lambda: emit_qk_unit(NPASS - 1, 1, "k")]}
            # likewise K for block 2 is first needed at pass-2 key tile 8
            # (job index 32): splice into pass 2 itself
            k2_sched = {20: [lambda: emit_qk_unit(NPASS - 2, 0, "k")],
                        24: [lambda: emit_qk_unit(NPASS - 2, 1, "k")]}

            # V units for block b are first used at pass b's own diagonal
            # tiles (i = 4b+k), so splice them into pass b shortly before use:
            # this feeds Act-bound attention stretches with PE-only work.
            v_sched = {p: {} for p in range(NPASS)}
            for p in range(1, NPASS):
                for k in range(4):
                    vidx = 1 + 3 * k
                    v_sched[p].setdefault(vidx, []).append(
                        lambda tt=4 * p + k: emit_v_unit(tt))
            # out-proj splice indices are appended at runtime (see loop below).

            po_cur = {}       # h -> psum tile for current pass
            osnaps = {}       # h -> osnap tile (pass-end normalization)
            recs = {}         # h -> [1,512] reciprocal tile

            rbs_pend = {}     # h -> broadcast reciprocal (emitted per job)

            def flush_norm(p, hs):
                """Deferred normalization part 2 for the given heads."""
                for h2 in hs:
                    nc.vector.tensor_mul(
                        OTs[64 * (h2 % 2) : 64 * (h2 % 2) + 64, h2 // 2,
                            ds(PW * p, 512)],
                        osnaps[h2][0:HDc, :],
                        rbs_pend[h2],
                    )
            ag_in_last = dramp.tile([NFT * 128, 512], BF16, tag="aginL",
                                    name="ag_in_last", bufs=1)
            ag_outA = dramp.tile([GRP * 128, 512], BF16, tag="agoutA",
                                 name="ag_outA", bufs=1)
            ag_outB = dramp.tile([GRP * 128, 512], BF16, tag="agoutB",
                                 name="ag_outB", bufs=1)
            OFL3 = ofp.tile([128, NDM, 512], BF16, tag="of", name="OFL3")

            for p in range(NPASS):
                npjobs = 4 * (4 * p + 4)
                base = sum(4 * (4 * q + 4) for q in range(p))
                # splice schedule: unit k fires before job index sched[k]
                units = list(pass_units[p])
                if p >= 1:
                    # out-proj of pass p-1: not before ~1/3 through the stream
                    # (AllGather must have completed)
                    op_units = [lambda pp_=p - 1, ttl=ttl: emit_oproj_unit(pp_, ttl)
                                for ttl in range(4)]
                else:
                    op_units = []
                qkv_sched = [max(1, (k + 1) * npjobs // (len(units) + 1))
                             for k in range(len(units))]
                op_start = max(1, 3 * npjobs // 4)
                op_sched = [min(npjobs - 1, op_start + k * max(1, npjobs // 8))
                            for k in range(len(op_units))]
                sched = {}
                for k, u in enumerate(units):
                    sched.setdefault(qkv_sched[k], []).append(u)
                for k, u in enumerate(op_units):
                    sched.setdefault(op_sched[k], []).append(u)
                for idx2, us in v_sched[p].items():
                    sched.setdefault(idx2, []).extend(us)
                if p == NPASS - 1:
                    for idx2, us in k3_sched.items():
                        sched.setdefault(idx2, []).extend(us)
                if p == NPASS - 2:
                    for idx2, us in k2_sched.items():
                        sched.setdefault(idx2, []).extend(us)


                for idx in range(npjobs):
                    gidx = base + idx
                    _, i, h = all_jobs[gidx]
                    hf, hp = h // 2, h % 2
                    ilast = 4 * p + 3
                    for u in sched.get(idx, []):
                        u()
                    if gidx == 0:
                        S_cur, al_cur = emit_scores(*all_jobs[0])
                        S_n1, al_n1 = emit_scores(*all_jobs[1])
                    S, al = S_cur, al_cur
                    S_cur, al_cur = S_n1, al_n1
                    if gidx + 2 < len(all_jobs):
                        S_n1, al_n1 = emit_scores(*all_jobs[gidx + 2])
                    # P = exp(S/32), causal-masked on the diagonal block
                    P = pbp.tile([128, 512], BF16, tag="ptile", name="P")
                    nc.scalar.activation(
                        P[:, ds(al, 512 - al)],
                        S[:, ds(al, 512 - al)],
                        mybir.ActivationFunctionType.Exp,
                        scale=float(cfg.scale),
                    )
                    if i >= 4 * p:  # diagonal block
                        nc.vector.tensor_mul(P[:, ds(al, 128)], P[:, ds(al, 128)],
                                             trimask)
                    if i == 0:
                        po_cur[h] = psum.tile([HDc + 1, 512], F32, tag="po",
                                              name="po")
                    po = po_cur[h]
                    # attnV accumulation, causally trimmed (partial-stop on the
                    # last key tile: the closing matmul covers only [al, 512))
                    nc.tensor.matmul(
                        po[:, ds(al, 512 - al)],
                        Vb[:, i, ds((HDc + 1) * h, HDc + 1)],
                        P[:, ds(al, 512 - al)],
                        start=(i == 0), stop=(i == ilast),
                    )
                    if i == ilast:
                        # normalization, part 1 (DVE only -- the PE part is
                        # deferred to the pass end so the in-order PE queue
                        # never parks on the reciprocal chain).  The mul
                        # reads po (PSUM) directly: rb is SBUF, so only one
                        # PSUM input -- no snapshot copy needed.
                        osrc = po
                        rec = nrm.tile([1, 512], F32R, tag="rec",
                                       name="rec", bufs=6)
                        with nc.allow_low_precision(reason="f32r is 32-bit"):
                            nc.vector.reciprocal(rec, osrc[HDc : HDc + 1, :])
                        recs[h] = rec
                        osnaps[h] = osrc
                        rbj = nrm.tile([64, 512], F32, tag="rb", name="rb",
                                       bufs=6)
                        nc.gpsimd.partition_broadcast(rbj, rec.bitcast(F32))
                        rbs_pend[h] = rbj
                    if p == NPASS - 1 and idx == npjobs // 2 - 1:
                        # heads 0,1 done: normalize them, store + gather
                        # feature tile 0 while heads 2,3 still attend
                        flush_norm(p, (0, 1))
                        nc.sync.dma_start(
                            ag_in_last[0:128, :], OTs[:, 0, ds(PW * p, 512)])
                        nc.gpsimd.collective_compute(
                            "AllGather", mybir.AluOpType.bypass,
                            ins=[ag_in_last[0:128, :].opt()],
                            outs=[ag_outA.opt()], replica_groups=groups)
                        agvA = ag_outA.rearrange("(r p) q -> r p q", p=128)
                        for c in (0, 2, 4, 6):
                            nc.sync.dma_start(OFL3[:, c, :], agvA[c // 2])
                if p == NPASS - 1:
                    flush_norm(p, (2, 3))
                    nc.sync.dma_start(
                        ag_in_last[128:256, :], OTs[:, 1, ds(PW * p, 512)])
                    nc.gpsimd.collective_compute(
                        "AllGather", mybir.AluOpType.bypass,
                        ins=[ag_in_last[128:256, :].opt()],
                        outs=[ag_outB.opt()], replica_groups=groups)
                    agvB = ag_outB.rearrange("(r p) q -> r p q", p=128)
                    for c in (1, 3, 5, 7):
                        nc.sync.dma_start(OFL3[:, c, :], agvB[c // 2])
                    OF_pending[p] = OFL3
                else:
                    flush_norm(p, tuple(range(HLOCc)))
                    # pass p complete: fire its AllGather (non-PE queues)
                    OF_pending[p] = emit_agather(p)

            # final pass's out-projection (unavoidable tail): chunk-outer so
            # each matmul only waits for its own gathered-feature chunk
            OFt = OF_pending[NPASS - 1]
            q0t = PW * (NPASS - 1)
            osbt = osbp.tile([128, 4, FLOCc], F32, tag="osb", name="osbt")
            poutt = [psum.tile([128, 512], F32, tag="sq", name="poutt")
                     for _ in range(4)]
            outvt = out[ds(q0t, 512), :].rearrange("(t p) f -> p t f", p=128)
            tail_cs = [0, 2, 4, 6, 1, 3, 5, 7]
            for ci, c in enumerate(tail_cs):
                for ttl in range(4):
                    nc.tensor.matmul(poutt[ttl][:, 0:FLOCc],
                                     OFt[:, c, ts(ttl, 128)], wob[:, c, :],
                                     start=(ci == 0),
                                     stop=(zb and ci == NDM - 1))
            for ttl in range(4):
                if not zb:
                    nc.tensor.matmul(poutt[ttl][:, 0:FLOCc], ones_r, bob,
                                     start=False, stop=True)
                nc.vector.tensor_copy(osbt[:, ttl, :], poutt[ttl][:, 0:FLOCc])
                if ttl == 1:
                    nc.sync.dma_start(outvt[:, 0:2, :], osbt[:, 0:2, :])
                elif ttl == 3:
                    nc.sync.dma_start(outvt[:, 2:4, :], osbt[:, 2:4, :])

        psum_cm.__exit__(None, None, None)


def make_program(cfg=None, groups=None, unroll=1):
    cfg = cfg or Cfg()
    groups = groups or REPLICA_GROUPS
    nc = bacc.Bacc("TRN2", target_bir_lowering=False, debug=False, num_devices=NCORES)
    x = nc.dram_tensor("x", [cfg.L, cfg.DM], F32, kind="ExternalInput").ap()
    wqkv = nc.dram_tensor("wqkv", [cfg.DM, 3 * cfg.FLOC], F32, kind="ExternalInput").ap()
    bq = nc.dram_tensor("bq", [cfg.FLOC], F32, kind="ExternalInput").ap()
    bk = nc.dram_tensor("bk", [cfg.FLOC], F32, kind="ExternalInput").ap()
    bv = nc.dram_tensor("bv", [cfg.FLOC], F32, kind="ExternalInput").ap()
    wo = nc.dram_tensor("wo", [cfg.DM, cfg.FLOC], F32, kind="ExternalInput").ap()
    bo = nc.dram_tensor("bo", [cfg.FLOC], F32, kind="ExternalInput").ap()
    out = nc.dram_tensor("out", [cfg.L, cfg.FLOC], F32, kind="ExternalOutput").ap()
    with tile.TileContext(nc) as tc:
        nc.tc = tc
        for _ in range(unroll):
            build_body(nc, cfg, x, wqkv, bq, bk, bv, wo, bo, out, groups)
    nc.compile()
    return nc


def shard_inputs(x, w_qkv, b_qkv, w_out, b_out, cfg=None):
    """Full inputs -> list of 8 per-core input dicts."""
    cfg = cfg or Cfg()
    FL = cfg.FLOC
    DMF = cfg.DM
    in_maps = []
    for c in range(NCORES):
        b, r = divmod(c, GRP)
        q0 = r * FL
        in_maps.append({
            "x": np.ascontiguousarray(x[b]),
            "wqkv": np.ascontiguousarray(np.concatenate([
                w_qkv[:, q0 : q0 + FL],
                w_qkv[:, DMF + q0 : DMF + q0 + FL],
                w_qkv[:, 2 * DMF + q0 : 2 * DMF + q0 + FL],
            ], axis=1)),
            "bq": np.ascontiguousarray(b_qkv[q0 : q0 + FL]),
            "bk": np.ascontiguousarray(b_qkv[DMF + q0 : DMF + q0 + FL]),
            "bv": np.ascontiguousarray(b_qkv[2 * DMF + q0 : 2 * DMF + q0 + FL]),
            "wo": np.ascontiguousarray(w_out[:, q0 : q0 + FL]),
            "bo": np.ascontiguousarray(b_out[q0 : q0 + FL]),
        })
    return in_maps


def gather_output(results, cfg=None):
    cfg = cfg or Cfg()
    FL = cfg.FLOC
    out = np.empty((BS, cfg.L, cfg.DM), np.float32)
    for c in range(NCORES):
        b, r = divmod(c, GRP)
        out[b, :, r * FL : (r + 1) * FL] = results[c]["out"]
    return out


_PROGRAMS = {}


def _get_program(zero_bias):
    if zero_bias not in _PROGRAMS:
        _PROGRAMS[zero_bias] = make_program(Cfg(zero_bias=zero_bias))
    return _PROGRAMS[zero_bias]


def kernel(x, w_qkv, b_qkv, w_out, b_out):
    x = np.asarray(x, np.float32)
    w_qkv = np.asarray(w_qkv, np.float32)
    b_qkv = np.asarray(b_qkv, np.float32)
    w_out = np.asarray(w_out, np.float32)
    b_out = np.asarray(b_out, np.float32)
    zero_bias = bool(not b_qkv.any() and not b_out.any())
    nc = _get_program(zero_bias)
    in_maps = shard_inputs(x, w_qkv, b_qkv, w_out, b_out)
    res = run_bass_kernel_spmd(nc, in_maps, list(range(NCORES)))
    return gather_output(res.results)
